# revision 8
# baseline (speedup 1.0000x reference)
"""Trainium2 Bass kernel for nn_EventSampler (Hawkes thinning sampler).

Math (per (b,l) row, fully independent):
  bound = 1.5 * max_s sum_m softplus(mu_m + alpha_m * gamma[type] * exp(-beta_m * t_s))
          over t_s in linspace(0,5,10); alpha,beta,gamma > 0 makes the max sit
          at t=0, so bound = 1.5 * sum_m softplus(mu_m + alpha_m*gamma[type]).
  exp_j = cumsum(-log1p(-e_unif) / bound)                       [E]
  intens[e] = sum_m softplus(mu_m + alpha_m*g*exp(-beta_m*exp_j[e]))
  accept[k,e] = u[k,e]*bound / intens[e] < 1
  res[k] = exp_j[first accepted e]  (0 if none), clamped to 1e5.

Reformulations used:
 1. exp_j is non-decreasing along e, so the first accepted exp_j equals the
    extremum over accepted e: a masked reduction, no gather. The device
    carries cums_neg = -bound*exp_j (raw segmented cumsum of log1p(-eu),
    unscaled); the host divides by -bound at decode time.
 2. Sign trick for mask+select: d = pH[e] - u*2^80 with pH = intens*2^80/bound
    (power-of-2 scaling keeps the sign decision at f32 fidelity); then
    val = min(d, cums_neg): accepted (d>0) contribute cums_neg in [-200, 0],
    rejected contribute d <= ~-1e16. max-reduce over e picks the FIRST
    accept (cums_neg is decreasing along e), or a <= -1e9 sentinel if none;
    the host decodes (min(-val/bound, 1e5)) and recomputes sentinel rows.
 3. Early exit: only the first E1=8 exponential draws are consulted
    (acceptance prob/draw is >=0.53); the ~3% of rows where some k has no
    accept within E1 are recomputed exactly on the host.
 4. Instruction-count-minimal program: this execution path charges a large
    fixed overhead per instruction (~40-80us), so all 8 row-segments (1024
    rows) of a core are processed by single big-AP instructions: the 8
    per-segment cumsums run as ONE segmented tensor_tensor_scan
    (state = mask*state + jraw, mask=0 at segment starts), and the whole
    accept/select/reduce over u is 3 instructions on [128, ~100, NT*E1] APs
    (k in the middle dim so the per-(segment,e) vectors broadcast with
    uniform 3D strides; walrus caps these ops at partition+2 free dims).
    Only Exp/Ln activations are used, steered to the shared
    natural_log_exp_and_others table set -> one act-table load total.
    Per rep: 2 DMAs (input triggered from the Activation queue so the Ln
    that consumes it follows on the same queue with no cross-engine sem;
    output from the otherwise-idle gpsimd queue so the Activation queue
    never stalls on tred before starting the next rep) + 4 activations
    + 9 DVE ops (+ ~4 scheduler semaphores) ~= 0.6-0.95 ms measured via
    the reps-slope; the baseline's ~250-instruction pipeline measured
    32-107 ms. Double-buffering (bufs=2) measures WORSE here (interleaved
    streams dispatch worse), so everything is single-buffered.

Sharding: data-parallel over the 8192 (b,l) rows, 1024 rows per core,
row r of a core lives at partition r%128, segment r//128.
"""

import sys
import functools
from contextlib import ExitStack

sys.path.insert(0, "/opt/trn_rl_repo")

import numpy as np

import concourse.bacc as bacc
import concourse.mybir as mybir
import concourse.tile as tile
from concourse.bass_utils import run_bass_kernel_spmd

# Steer the act-table chooser to the set containing BOTH exp and ln
# (natural_log_exp_and_others) so the per-rep Ln->Exp->Exp->Ln sequence needs
# one table load total instead of two reloads per rep. Set indices are left
# untouched (only exp/ln are hidden from the single-function sets), so the
# emitted act_func_set_id still refers to the true act_info.json entry.
_orig_get_act_tables = bacc.get_activation_tables


def _patched_get_act_tables(arch):
    tabs = _orig_get_act_tables(arch)
    both = {
        name
        for name, fns in tabs.items()
        if mybir.ActivationFunctionType.Exp in fns
        and mybir.ActivationFunctionType.Ln in fns
    }
    if both:
        for name, fns in tabs.items():
            if name not in both:
                fns.discard(mybir.ActivationFunctionType.Exp)
                fns.discard(mybir.ActivationFunctionType.Ln)
    return tabs


bacc.get_activation_tables = _patched_get_act_tables

B, L, E, K, M, NTYPES = 4, 2048, 100, 100, 10, 10
OVER_SAMPLE_RATE = 1.5
DTIME_MAX = 5.0
NUM_SAMPLES_BOUNDARY = 10

NCORES = 8
ROWS = B * L            # 8192 independent (b,l) rows
RPC = ROWS // NCORES    # 1024 rows per core
PT = 128                # partitions
NT = RPC // PT          # 8 row-segments per core
E1 = 8                  # draws consulted on device; rows needing more (~3%)
                        # are recomputed exactly on the host
TE = NT * E1            # flattened (segment, e) inner dim = 128
BIGF = 1.0e9            # accept/reject sentinel threshold on host
HUGE = 2.0 ** 80        # exact power-of-2 scale: rejects land >= ~1e16

F32 = mybir.dt.float32
F16 = mybir.dt.float16
ALU = mybir.AluOpType
ACTF = mybir.ActivationFunctionType
AX = mybir.AxisListType


def _build(reps: int = 1):
    """Build the per-core Bass program (reps>1 repeats compute, for timing)."""
    nc = bacc.Bacc()

    ui = nc.dram_tensor("ui", [PT, (K + 1) * TE], F32, kind="ExternalInput")
    tq = nc.dram_tensor("tq", [RPC], F32, kind="ExternalInput")
    mu = nc.dram_tensor("mu", [M], F32, kind="ExternalInput")
    al = nc.dram_tensor("al", [M], F32, kind="ExternalInput")
    be = nc.dram_tensor("be", [M], F32, kind="ExternalInput")
    ga = nc.dram_tensor("ga", [NTYPES], F32, kind="ExternalInput")
    ar = nc.dram_tensor("ar", [NTYPES], F32, kind="ExternalInput")
    ro = nc.dram_tensor("ro", [PT, K * NT], F16, kind="ExternalOutput")

    with tile.TileContext(nc) as tc:
        with (
            tc.tile_pool(name="const", bufs=1) as pc,
            tc.tile_pool(name="work", bufs=1) as pw,
            tc.tile_pool(name="big", bufs=1) as pb,
        ):
            # ---- phase 0 (once per call): per-row constants ------------------
            tga = pc.tile([PT, NTYPES], F32)
            tmu = pc.tile([PT, M], F32)
            tal = pc.tile([PT, M], F32)
            tbe = pc.tile([PT, M], F32)
            tar = pc.tile([PT, NTYPES], F32)
            ttq = pc.tile([PT, NT], F32)
            nc.sync.dma_start(tga[:], ga[:].unsqueeze(0).broadcast_to([PT, NTYPES]))
            nc.sync.dma_start(tmu[:], mu[:].unsqueeze(0).broadcast_to([PT, M]))
            nc.sync.dma_start(tal[:], al[:].unsqueeze(0).broadcast_to([PT, M]))
            nc.sync.dma_start(tbe[:], be[:].unsqueeze(0).broadcast_to([PT, M]))
            nc.sync.dma_start(tar[:], ar[:].unsqueeze(0).broadcast_to([PT, NTYPES]))
            nc.sync.dma_start(ttq[:], tq[:].rearrange("(t p) -> p t", p=PT))

            # one-hot gamma gather, all segments at once: g[p,t]
            toh = pw.tile([PT, NT, NTYPES], F32, tag="toh")
            nc.vector.tensor_tensor(
                toh[:],
                tar[:].unsqueeze(1).broadcast_to([PT, NT, NTYPES]),
                ttq[:].unsqueeze(2).broadcast_to([PT, NT, NTYPES]),
                op=ALU.is_equal,
            )
            tgm = pw.tile([PT, NT, NTYPES], F32, tag="tgm")
            nc.vector.tensor_tensor(
                tgm[:],
                toh[:],
                tga[:].unsqueeze(1).broadcast_to([PT, NT, NTYPES]),
                op=ALU.mult,
            )
            g_all = pc.tile([PT, NT], F32)
            nc.vector.tensor_reduce(g_all[:], tgm[:], axis=AX.X, op=ALU.add)

            # ag[p,t,m] = alpha_m * g[p,t]; bound = 1.5*sum_m softplus(mu+ag)
            ag_all = pc.tile([PT, NT, M], F32)
            nc.vector.tensor_tensor(
                ag_all[:],
                tal[:].unsqueeze(1).broadcast_to([PT, NT, M]),
                g_all[:].unsqueeze(2).broadcast_to([PT, NT, M]),
                op=ALU.mult,
            )
            tzb = pw.tile([PT, NT, M], F32, tag="tzb")
            nc.vector.tensor_tensor(
                tzb[:],
                ag_all[:],
                tmu[:].unsqueeze(1).broadcast_to([PT, NT, M]),
                op=ALU.add,
            )
            teb = pw.tile([PT, NT, M], F32, tag="teb")
            nc.scalar.activation(
                teb[:].rearrange("p t m -> p (t m)"),
                tzb[:].rearrange("p t m -> p (t m)"),
                ACTF.Exp,
            )
            tsb = pw.tile([PT, NT, M], F32, tag="tsb")
            nc.scalar.activation(
                tsb[:].rearrange("p t m -> p (t m)"),
                teb[:].rearrange("p t m -> p (t m)"),
                ACTF.Ln,
                bias=1.0,
            )
            tbs = pw.tile([PT, NT], F32, tag="tbs")
            nc.vector.tensor_reduce(tbs[:], tsb[:], axis=AX.X, op=ALU.add)
            bound = pc.tile([PT, NT], F32)
            nc.vector.tensor_scalar_mul(bound[:], tbs[:], OVER_SAMPLE_RATE)
            trb = pc.tile([PT, NT], F32)
            nc.vector.reciprocal(trb[:], bound[:])
            nrbH = pc.tile([PT, NT], F32)      # 2^80/bound (threshold scale)
            nc.vector.tensor_scalar_mul(nrbH[:], trb[:], HUGE)

            # bebx[p,(t,e),m] = beta_m/bound[p,t] expanded over e (free here;
            # lets the per-rep intensity input be cums_neg*bebx in one 3D TT)
            bebx = pc.tile([PT, TE, M], F32)
            for t in range(NT):
                nc.vector.tensor_scalar_mul(
                    bebx[:, t * E1 : (t + 1) * E1, :],
                    tbe[:].unsqueeze(1).broadcast_to([PT, E1, M]),
                    trb[:, t : t + 1],
                )

            # ag expanded over e (free instructions here; keeps rep ops 3D):
            # agx[p, (t,e), m] = ag[p, t, m]
            agx = pc.tile([PT, TE, M], F32)
            for t in range(NT):
                nc.vector.tensor_scalar_mul(
                    agx[:, t * E1 : (t + 1) * E1, :],
                    ag_all[:, t, :].unsqueeze(1).broadcast_to([PT, E1, M]),
                    1.0,
                )

            # segmented-scan mask: 0 at e==0 of each segment, 1 elsewhere
            mask = pc.tile([PT, NT, E1], F32)
            nc.vector.memset(mask[:], 1.0)
            nc.vector.memset(mask[:, :, 0:1], 0.0)

            # ---- per-rep pipeline (For_i_pipelined hardware loop) -----------
            # Per-STATIC-instruction dispatch costs ~60us on this execution
            # path but re-execution inside a hardware loop runs at true
            # engine speed, so the steady-state rep cost is the engine-level
            # cost only. 3 stages (load / compute / store), unroll=2: the
            # input DMA (split over the idle sync+tensor queues) overlaps the
            # act+vector compute of the previous rep; act-engine tiles are
            # double-buffered so the act engine runs one rep ahead of the
            # vector engine. Big accept tiles are single-buffered (vector is
            # serial anyway) and the min() is done in place.
            tick = [0]

            def _load(pipe, iv):
                tui = pipe.intermediate_tile([PT, K + 1, TE], F32, name="tui")
                flat = tui[:].rearrange("p k f -> p (k f)")
                half = (K + 1) * TE // 2
                nc.sync.dma_start(flat[:, 0:half], ui[:, 0:half])
                nc.scalar.dma_start(
                    flat[:, half : (K + 1) * TE], ui[:, half : (K + 1) * TE]
                )
                return tui

            def _compute(pipe, iv, tui):
                sfx = tick[0] % 2
                tick[0] += 1
                u_v = tui[:, 0:K, :]                          # [PT,K,TE]
                eu_v = tui[:, K, :]                           # [PT,TE]

                # jraw = log1p(-eu)  (<= 0; this is -e of the reference)
                jraw = pw.tile([PT, NT, E1], F32, tag=f"jraw{sfx}")
                nc.scalar.activation(
                    jraw[:].rearrange("p t e -> p (t e)"),
                    eu_v,
                    ACTF.Ln,
                    bias=1.0,
                    scale=-1.0,
                )
                # cums_neg = -bound*exp_j: segmented cumsum of jraw in ONE
                # scan (state = mask*state + jraw); host divides by -bound
                ej = pw.tile([PT, NT, E1], F32, tag=f"ej{sfx}")
                nc.vector.tensor_tensor_scan(
                    ej[:].rearrange("p t e -> p (t e)"),
                    mask[:].rearrange("p t e -> p (t e)"),
                    jraw[:].rearrange("p t e -> p (t e)"),
                    0.0,
                    op0=ALU.mult,
                    op1=ALU.add,
                )
                ej2 = ej[:].rearrange("p t e -> p (t e)")      # [PT,TE] cums_neg

                # intens[p,(t,e)] = sum_m softplus(mu + ag*exp(-beta*ej))
                # where -beta*exp_j = cums_neg*beta/bound = cums_neg*bebx
                txp = pw.tile([PT, TE, M], F32, tag=f"txp{sfx}")
                nc.vector.tensor_tensor(
                    txp[:],
                    ej2.unsqueeze(2).broadcast_to([PT, TE, M]),
                    bebx[:],
                    op=ALU.mult,
                )
                tem = pw.tile([PT, TE, M], F32, tag=f"tem{sfx}")
                nc.scalar.activation(
                    tem[:].rearrange("p f m -> p (f m)"),
                    txp[:].rearrange("p f m -> p (f m)"),
                    ACTF.Exp,
                )
                tin1 = pw.tile([PT, TE, M], F32, tag=f"tin1{sfx}")
                nc.vector.tensor_tensor(tin1[:], tem[:], agx[:], op=ALU.mult)
                tin2 = pw.tile([PT, TE, M], F32, tag=f"tin2{sfx}")
                nc.vector.tensor_tensor(
                    tin2[:],
                    tin1[:],
                    tmu[:].unsqueeze(1).broadcast_to([PT, TE, M]),
                    op=ALU.add,
                )
                te4 = pw.tile([PT, TE, M], F32, tag=f"te4{sfx}")
                nc.scalar.activation(
                    te4[:].rearrange("p f m -> p (f m)"),
                    tin2[:].rearrange("p f m -> p (f m)"),
                    ACTF.Exp,
                )
                tsp = pw.tile([PT, TE, M], F32, tag=f"tsp{sfx}")
                nc.scalar.activation(
                    tsp[:].rearrange("p f m -> p (f m)"),
                    te4[:].rearrange("p f m -> p (f m)"),
                    ACTF.Ln,
                    bias=1.0,
                )
                tint = pw.tile([PT, NT, E1], F32, tag=f"tint{sfx}")
                nc.vector.tensor_reduce(
                    tint[:].rearrange("p t e -> p (t e)"),
                    tsp[:],
                    axis=AX.X,
                    op=ALU.add,
                )
                # pH = intens * 2^80 / bound
                pH = pw.tile([PT, NT, E1], F32, tag=f"pH{sfx}")
                nc.vector.tensor_tensor(
                    pH[:],
                    tint[:],
                    nrbH[:].unsqueeze(2).broadcast_to([PT, NT, E1]),
                    op=ALU.mult,
                )
                pH2 = pH[:].rearrange("p t e -> p (t e)")      # [PT,TE]

                # fp16 copy of cums_neg for the 2x-throughput accept ops
                ejh = pw.tile([PT, NT, E1], F16, tag=f"ejh{sfx}")
                nc.vector.tensor_scalar_mul(
                    ejh[:].rearrange("p t e -> p (t e)"), ej2, 1.0
                )
                ejh2 = ejh[:].rearrange("p t e -> p (t e)")

                # accept/select/reduce over ALL of u in 3 instructions:
                # d2 = pH - u*2^80 (accept>0, reject<0; |d2|>=4.8e16 always,
                # so the fp16 store saturates to EXACTLY +-inf: the fp16
                # value carries just the accept bit);
                # v = min(d2, cums_neg) in place at fp16 2x rate: accept ->
                # cums_neg in [-200,0], reject -> -inf; max-reduce picks the
                # FIRST accept (cums_neg is decreasing along e), or -inf.
                td = pb.tile([PT, K, TE], F16, tag="td")
                nc.vector.scalar_tensor_tensor(
                    td[:],
                    u_v,
                    -HUGE,
                    pH2.unsqueeze(1).broadcast_to([PT, K, TE]),
                    op0=ALU.mult,
                    op1=ALU.add,
                )
                nc.vector.tensor_tensor(
                    td[:],
                    td[:],
                    ejh2.unsqueeze(1).broadcast_to([PT, K, TE]),
                    op=ALU.min,
                )
                tred = pipe.intermediate_tile([PT, K * NT], F16, name="tred")
                nc.vector.tensor_reduce(
                    tred[:],
                    td[:].rearrange("p k (t e) -> p (k t) e", t=NT),
                    axis=AX.X,
                    op=ALU.max,
                )
                return tred

            def _store(pipe, iv, tred):
                nc.gpsimd.dma_start(ro[:, :], tred[:])

            tc.For_i_pipelined(
                [_load, _compute, _store], 0, reps, unroll=4, staged_num_bufs=2
            )

    nc.compile()
    return nc


@functools.lru_cache(maxsize=4)
def _built(reps: int):
    return _build(reps=reps)


def _host_rows(rows, e_unif, u, g_rows, muf, alf, bef):
    """Reference-faithful numpy fallback for rows not resolved within E1.

    Vectorized over all fallback rows at once (a per-row Python loop costs
    ~70 ms for the typical ~240 rows; this runs in a few ms)."""
    tn = np.linspace(0.0, DTIME_MAX, NUM_SAMPLES_BOUNDARY).astype(np.float32)
    g = np.asarray(g_rows, np.float32)[:, None, None]                 # [N,1,1]
    zb = muf[None, None, :] + alf[None, None, :] * g * np.exp(
        -bef[None, None, :] * tn[None, :, None]
    )                                                                 # [N,S,M]
    bound = (
        np.log1p(np.exp(zb)).sum(-1).max(-1) * np.float32(OVER_SAMPLE_RATE)
    ).astype(np.float32)                                              # [N]
    e = -np.log1p(-e_unif[rows])                                      # [N,E]
    expj = np.cumsum(e / bound[:, None], axis=-1).astype(np.float32)  # [N,E]
    zi = muf[None, None, :] + alf[None, None, :] * g * np.exp(
        -bef[None, None, :] * expj[:, :, None]
    )                                                                 # [N,E,M]
    it = np.log1p(np.exp(zi)).sum(-1).astype(np.float32)              # [N,E]
    crit = u[rows] * bound[:, None, None] / it[:, None, :]            # [N,K,E]
    mask = crit < 1.0
    anya = mask.any(-1)
    idx = mask.argmax(-1)                                             # [N,K]
    ej_b = np.broadcast_to(expj[:, None, :], mask.shape)
    res = np.where(
        anya, np.take_along_axis(ej_b, idx[..., None], 2)[..., 0], np.float32(0.0)
    )
    return np.minimum(res, np.float32(1.0e5)).astype(np.float32)


def kernel(
    time_seqs,
    time_delta_seqs,
    type_seqs,
    e_unif,
    u,
    mu,
    alpha,
    beta,
    gamma,
    num_sample,
    _reps: int = 1,
):
    e_unif = np.asarray(e_unif, dtype=np.float32).reshape(ROWS, E)
    u = np.asarray(u, dtype=np.float32).reshape(ROWS, K, E)
    tqf = np.ascontiguousarray(np.asarray(type_seqs).astype(np.float32)).reshape(ROWS)
    muf = np.ascontiguousarray(np.asarray(mu, dtype=np.float32))
    alf = np.ascontiguousarray(np.asarray(alpha, dtype=np.float32))
    bef = np.ascontiguousarray(np.asarray(beta, dtype=np.float32))
    gaf = np.ascontiguousarray(np.asarray(gamma, dtype=np.float32))
    arf = np.arange(NTYPES, dtype=np.float32)

    nc = _built(_reps)
    in_maps = []
    for c in range(NCORES):
        rs = slice(c * RPC, (c + 1) * RPC)
        # pack u[:, :, :E1] and eu[:, :E1]: row t*PT+p -> (p, ., t, e)
        u_part = (
            u[rs, :, :E1].reshape(NT, PT, K, E1).transpose(1, 2, 0, 3)
        )  # [PT,K,NT,E1]
        eu_part = (
            e_unif[rs, :E1].reshape(NT, PT, E1).transpose(1, 0, 2)
        )  # [PT,NT,E1]
        ui = np.concatenate(
            [u_part.reshape(PT, K, TE), eu_part.reshape(PT, 1, TE)], axis=1
        )
        in_maps.append(
            {
                "ui": np.ascontiguousarray(ui.reshape(PT, (K + 1) * TE)),
                "tq": tqf[rs],
                "mu": muf,
                "al": alf,
                "be": bef,
                "ga": gaf,
                "ar": arf,
            }
        )
    out = run_bass_kernel_spmd(nc, in_maps, core_ids=list(range(NCORES)))
    ro = np.concatenate(
        [
            out.results[c]["ro"]
            .reshape(PT, K, NT)
            .transpose(2, 0, 1)
            .reshape(RPC, K)
            for c in range(NCORES)
        ],
        axis=0,
    )  # [ROWS, K] raw max-values: -bound*exp_j at first accept, or <=-1e16

    # decode: exp_j = -val/bound (bound recomputed on host, matches device
    # within f32 rounding; only scales the output)
    g_h = gaf[tqf.astype(np.int64)]                                   # [ROWS]
    tn_h = np.linspace(0.0, DTIME_MAX, NUM_SAMPLES_BOUNDARY).astype(np.float32)
    z_h = muf[None, None, :] + alf[None, None, :] * g_h[:, None, None] * np.exp(
        -bef[None, None, :] * tn_h[None, :, None]
    )
    bound_h = (
        np.log1p(np.exp(z_h)).sum(-1).max(-1) * np.float32(OVER_SAMPLE_RATE)
    ).astype(np.float32)                                              # [ROWS]

    res = np.minimum(-ro / bound_h[:, None], np.float32(1.0e5)).astype(np.float32)
    bad_rows = np.nonzero((ro <= -BIGF).any(axis=1))[0]
    if len(bad_rows):
        res[bad_rows] = _host_rows(
            bad_rows, e_unif, u, g_h[bad_rows], muf, alf, bef
        )

    res = res.reshape(B, L, K)
    weights = np.full((B, L, K), 1.0 / float(num_sample), dtype=np.float32)
    return res, weights



# revision 9
# speedup vs baseline: 1.3172x; 1.3172x over previous
"""Trainium2 Bass kernel for nn_EventSampler (Hawkes thinning sampler).

Math (per (b,l) row, fully independent):
  bound = 1.5 * max_s sum_m softplus(mu_m + alpha_m * gamma[type] * exp(-beta_m * t_s))
          over t_s in linspace(0,5,10); alpha,beta,gamma > 0 makes the max sit
          at t=0, so bound = 1.5 * sum_m softplus(mu_m + alpha_m*gamma[type]).
  exp_j = cumsum(-log1p(-e_unif) / bound)                       [E]
  intens[e] = sum_m softplus(mu_m + alpha_m*g*exp(-beta_m*exp_j[e]))
  accept[k,e] = u[k,e]*bound / intens[e] < 1
  res[k] = exp_j[first accepted e]  (0 if none), clamped to 1e5.

Reformulations used:
 1. exp_j is non-decreasing along e, so the first accepted exp_j equals the
    extremum over accepted e: a masked reduction, no gather. The device
    carries cums_neg = -bound*exp_j (raw segmented cumsum of log1p(-eu),
    unscaled); the host divides by -bound at decode time.
 2. Sign trick for mask+select: d = pH[e] - u*2^80 with pH = intens*2^80/bound
    (power-of-2 scaling keeps the sign decision at f32 fidelity); then
    val = min(d, cums_neg): accepted (d>0) contribute cums_neg in [-200, 0],
    rejected contribute d <= ~-1e16. max-reduce over e picks the FIRST
    accept (cums_neg is decreasing along e), or a <= -1e9 sentinel if none;
    the host decodes (min(-val/bound, 1e5)) and recomputes sentinel rows.
 3. Early exit: only the first E1=8 exponential draws are consulted
    (acceptance prob/draw is >=0.53); the ~3% of rows where some k has no
    accept within E1 are recomputed exactly on the host.
 4. Instruction-count-minimal program: this execution path charges a large
    fixed overhead per instruction (~40-80us), so all 8 row-segments (1024
    rows) of a core are processed by single big-AP instructions: the 8
    per-segment cumsums run as ONE segmented tensor_tensor_scan
    (state = mask*state + jraw, mask=0 at segment starts), and the whole
    accept/select/reduce over u is 3 instructions on [128, ~100, NT*E1] APs
    (k in the middle dim so the per-(segment,e) vectors broadcast with
    uniform 3D strides; walrus caps these ops at partition+2 free dims).
    Only Exp/Ln activations are used, steered to the shared
    natural_log_exp_and_others table set -> one act-table load total.
    Per rep: 2 DMAs (input triggered from the Activation queue so the Ln
    that consumes it follows on the same queue with no cross-engine sem;
    output from the otherwise-idle gpsimd queue so the Activation queue
    never stalls on tred before starting the next rep) + 4 activations
    + 9 DVE ops (+ ~4 scheduler semaphores) ~= 0.6-0.95 ms measured via
    the reps-slope; the baseline's ~250-instruction pipeline measured
    32-107 ms. Double-buffering (bufs=2) measures WORSE here (interleaved
    streams dispatch worse), so everything is single-buffered.

Sharding: data-parallel over the 8192 (b,l) rows, 1024 rows per core,
row r of a core lives at partition r%128, segment r//128.
"""

import sys
import functools
from contextlib import ExitStack

sys.path.insert(0, "/opt/trn_rl_repo")

import numpy as np

import concourse.bacc as bacc
import concourse.mybir as mybir
import concourse.tile as tile
from concourse.bass_utils import run_bass_kernel_spmd

# Steer the act-table chooser to the set containing BOTH exp and ln
# (natural_log_exp_and_others) so the per-rep Ln->Exp->Exp->Ln sequence needs
# one table load total instead of two reloads per rep. Set indices are left
# untouched (only exp/ln are hidden from the single-function sets), so the
# emitted act_func_set_id still refers to the true act_info.json entry.
_orig_get_act_tables = bacc.get_activation_tables


def _patched_get_act_tables(arch):
    tabs = _orig_get_act_tables(arch)
    both = {
        name
        for name, fns in tabs.items()
        if mybir.ActivationFunctionType.Exp in fns
        and mybir.ActivationFunctionType.Ln in fns
    }
    if both:
        for name, fns in tabs.items():
            if name not in both:
                fns.discard(mybir.ActivationFunctionType.Exp)
                fns.discard(mybir.ActivationFunctionType.Ln)
    return tabs


bacc.get_activation_tables = _patched_get_act_tables

B, L, E, K, M, NTYPES = 4, 2048, 100, 100, 10, 10
OVER_SAMPLE_RATE = 1.5
DTIME_MAX = 5.0
NUM_SAMPLES_BOUNDARY = 10

NCORES = 8
ROWS = B * L            # 8192 independent (b,l) rows
RPC = ROWS // NCORES    # 1024 rows per core
PT = 128                # partitions
NT = RPC // PT          # 8 row-segments per core
E1 = 8                  # draws consulted on device; rows needing more (~3%)
                        # are recomputed exactly on the host
TE = NT * E1            # flattened (segment, e) inner dim = 128
BIGF = 1.0e9            # accept/reject sentinel threshold on host
HUGE = 2.0 ** 80        # exact power-of-2 scale: rejects land >= ~1e16

F32 = mybir.dt.float32
F16 = mybir.dt.float16
ALU = mybir.AluOpType
ACTF = mybir.ActivationFunctionType
AX = mybir.AxisListType


def _build(reps: int = 1):
    """Build the per-core Bass program (reps>1 repeats compute, for timing)."""
    nc = bacc.Bacc()

    ui = nc.dram_tensor("ui", [PT, (K + 1) * TE], F32, kind="ExternalInput")
    tq = nc.dram_tensor("tq", [RPC], F32, kind="ExternalInput")
    mu = nc.dram_tensor("mu", [M], F32, kind="ExternalInput")
    al = nc.dram_tensor("al", [M], F32, kind="ExternalInput")
    be = nc.dram_tensor("be", [M], F32, kind="ExternalInput")
    ga = nc.dram_tensor("ga", [NTYPES], F32, kind="ExternalInput")
    ar = nc.dram_tensor("ar", [NTYPES], F32, kind="ExternalInput")
    ro = nc.dram_tensor("ro", [PT, K * NT], F16, kind="ExternalOutput")

    with tile.TileContext(nc) as tc:
        with (
            tc.tile_pool(name="const", bufs=1) as pc,
            tc.tile_pool(name="work", bufs=1) as pw,
            tc.tile_pool(name="big", bufs=1) as pb,
        ):
            # ---- phase 0 (once per call): per-row constants ------------------
            tga = pc.tile([PT, NTYPES], F32)
            tmu = pc.tile([PT, M], F32)
            tal = pc.tile([PT, M], F32)
            tbe = pc.tile([PT, M], F32)
            tar = pc.tile([PT, NTYPES], F32)
            ttq = pc.tile([PT, NT], F32)
            nc.sync.dma_start(tga[:], ga[:].unsqueeze(0).broadcast_to([PT, NTYPES]))
            nc.sync.dma_start(tmu[:], mu[:].unsqueeze(0).broadcast_to([PT, M]))
            nc.sync.dma_start(tal[:], al[:].unsqueeze(0).broadcast_to([PT, M]))
            nc.sync.dma_start(tbe[:], be[:].unsqueeze(0).broadcast_to([PT, M]))
            nc.sync.dma_start(tar[:], ar[:].unsqueeze(0).broadcast_to([PT, NTYPES]))
            nc.sync.dma_start(ttq[:], tq[:].rearrange("(t p) -> p t", p=PT))

            # one-hot gamma gather, all segments at once: g[p,t]
            toh = pw.tile([PT, NT, NTYPES], F32, tag="toh")
            nc.vector.tensor_tensor(
                toh[:],
                tar[:].unsqueeze(1).broadcast_to([PT, NT, NTYPES]),
                ttq[:].unsqueeze(2).broadcast_to([PT, NT, NTYPES]),
                op=ALU.is_equal,
            )
            tgm = pw.tile([PT, NT, NTYPES], F32, tag="tgm")
            nc.vector.tensor_tensor(
                tgm[:],
                toh[:],
                tga[:].unsqueeze(1).broadcast_to([PT, NT, NTYPES]),
                op=ALU.mult,
            )
            g_all = pc.tile([PT, NT], F32)
            nc.vector.tensor_reduce(g_all[:], tgm[:], axis=AX.X, op=ALU.add)

            # ag[p,t,m] = alpha_m * g[p,t]; bound = 1.5*sum_m softplus(mu+ag)
            ag_all = pc.tile([PT, NT, M], F32)
            nc.vector.tensor_tensor(
                ag_all[:],
                tal[:].unsqueeze(1).broadcast_to([PT, NT, M]),
                g_all[:].unsqueeze(2).broadcast_to([PT, NT, M]),
                op=ALU.mult,
            )
            tzb = pw.tile([PT, NT, M], F32, tag="tzb")
            nc.vector.tensor_tensor(
                tzb[:],
                ag_all[:],
                tmu[:].unsqueeze(1).broadcast_to([PT, NT, M]),
                op=ALU.add,
            )
            teb = pw.tile([PT, NT, M], F32, tag="teb")
            nc.scalar.activation(
                teb[:].rearrange("p t m -> p (t m)"),
                tzb[:].rearrange("p t m -> p (t m)"),
                ACTF.Exp,
            )
            tsb = pw.tile([PT, NT, M], F32, tag="tsb")
            nc.scalar.activation(
                tsb[:].rearrange("p t m -> p (t m)"),
                teb[:].rearrange("p t m -> p (t m)"),
                ACTF.Ln,
                bias=1.0,
            )
            tbs = pw.tile([PT, NT], F32, tag="tbs")
            nc.vector.tensor_reduce(tbs[:], tsb[:], axis=AX.X, op=ALU.add)
            bound = pc.tile([PT, NT], F32)
            nc.vector.tensor_scalar_mul(bound[:], tbs[:], OVER_SAMPLE_RATE)
            trb = pc.tile([PT, NT], F32)
            nc.vector.reciprocal(trb[:], bound[:])
            nrbH = pc.tile([PT, NT], F32)      # 2^80/bound (threshold scale)
            nc.vector.tensor_scalar_mul(nrbH[:], trb[:], HUGE)

            # bebx[p,(t,e),m] = beta_m/bound[p,t] expanded over e (free here;
            # lets the per-rep intensity input be cums_neg*bebx in one 3D TT)
            bebx = pc.tile([PT, TE, M], F32)
            for t in range(NT):
                nc.vector.tensor_scalar_mul(
                    bebx[:, t * E1 : (t + 1) * E1, :],
                    tbe[:].unsqueeze(1).broadcast_to([PT, E1, M]),
                    trb[:, t : t + 1],
                )

            # ag expanded over e (free instructions here; keeps rep ops 3D):
            # agx[p, (t,e), m] = ag[p, t, m]
            agx = pc.tile([PT, TE, M], F32)
            for t in range(NT):
                nc.vector.tensor_scalar_mul(
                    agx[:, t * E1 : (t + 1) * E1, :],
                    ag_all[:, t, :].unsqueeze(1).broadcast_to([PT, E1, M]),
                    1.0,
                )

            # segmented-scan mask: 0 at e==0 of each segment, 1 elsewhere
            mask = pc.tile([PT, NT, E1], F32)
            nc.vector.memset(mask[:], 1.0)
            nc.vector.memset(mask[:, :, 0:1], 0.0)

            # ---- per-rep pipeline (For_i_pipelined hardware loop) -----------
            # Per-STATIC-instruction dispatch costs ~60us on this execution
            # path but re-execution inside a hardware loop runs at true
            # engine speed, so the steady-state rep cost is the engine-level
            # cost only. 3 stages (load / compute / store), unroll=2: the
            # input DMA (split over the idle sync+tensor queues) overlaps the
            # act+vector compute of the previous rep; act-engine tiles are
            # double-buffered so the act engine runs one rep ahead of the
            # vector engine. Big accept tiles are single-buffered (vector is
            # serial anyway) and the min() is done in place.
            tick = [0]

            def _load(pipe, iv):
                tui = pipe.intermediate_tile([PT, K + 1, TE], F32, name="tui")
                flat = tui[:].rearrange("p k f -> p (k f)")
                half = (K + 1) * TE // 2
                nc.sync.dma_start(flat[:, 0:half], ui[:, 0:half])
                nc.scalar.dma_start(
                    flat[:, half : (K + 1) * TE], ui[:, half : (K + 1) * TE]
                )
                return tui

            def _compute(pipe, iv, tui):
                sfx = tick[0] % 2
                tick[0] += 1
                u_v = tui[:, 0:K, :]                          # [PT,K,TE]
                eu_v = tui[:, K, :]                           # [PT,TE]

                # jraw = log1p(-eu)  (<= 0; this is -e of the reference)
                jraw = pw.tile([PT, NT, E1], F32, tag=f"jraw{sfx}")
                nc.scalar.activation(
                    jraw[:].rearrange("p t e -> p (t e)"),
                    eu_v,
                    ACTF.Ln,
                    bias=1.0,
                    scale=-1.0,
                )
                # cums_neg = -bound*exp_j: segmented cumsum of jraw in ONE
                # scan (state = mask*state + jraw); host divides by -bound
                ej = pw.tile([PT, NT, E1], F32, tag=f"ej{sfx}")
                nc.vector.tensor_tensor_scan(
                    ej[:].rearrange("p t e -> p (t e)"),
                    mask[:].rearrange("p t e -> p (t e)"),
                    jraw[:].rearrange("p t e -> p (t e)"),
                    0.0,
                    op0=ALU.mult,
                    op1=ALU.add,
                )
                ej2 = ej[:].rearrange("p t e -> p (t e)")      # [PT,TE] cums_neg

                # intens[p,(t,e)] = sum_m softplus(mu + ag*exp(-beta*ej))
                # where -beta*exp_j = cums_neg*beta/bound = cums_neg*bebx
                txp = pw.tile([PT, TE, M], F32, tag=f"txp{sfx}")
                nc.vector.tensor_tensor(
                    txp[:],
                    ej2.unsqueeze(2).broadcast_to([PT, TE, M]),
                    bebx[:],
                    op=ALU.mult,
                )
                tem = pw.tile([PT, TE, M], F32, tag=f"tem{sfx}")
                nc.scalar.activation(
                    tem[:].rearrange("p f m -> p (f m)"),
                    txp[:].rearrange("p f m -> p (f m)"),
                    ACTF.Exp,
                )
                tin1 = pw.tile([PT, TE, M], F32, tag=f"tin1{sfx}")
                nc.vector.tensor_tensor(tin1[:], tem[:], agx[:], op=ALU.mult)
                tin2 = pw.tile([PT, TE, M], F32, tag=f"tin2{sfx}")
                nc.vector.tensor_tensor(
                    tin2[:],
                    tin1[:],
                    tmu[:].unsqueeze(1).broadcast_to([PT, TE, M]),
                    op=ALU.add,
                )
                te4 = pw.tile([PT, TE, M], F32, tag=f"te4{sfx}")
                nc.scalar.activation(
                    te4[:].rearrange("p f m -> p (f m)"),
                    tin2[:].rearrange("p f m -> p (f m)"),
                    ACTF.Exp,
                )
                tsp = pw.tile([PT, TE, M], F32, tag=f"tsp{sfx}")
                nc.scalar.activation(
                    tsp[:].rearrange("p f m -> p (f m)"),
                    te4[:].rearrange("p f m -> p (f m)"),
                    ACTF.Ln,
                    bias=1.0,
                )
                tint = pw.tile([PT, NT, E1], F32, tag=f"tint{sfx}")
                nc.vector.tensor_reduce(
                    tint[:].rearrange("p t e -> p (t e)"),
                    tsp[:],
                    axis=AX.X,
                    op=ALU.add,
                )
                # pH = intens * 2^80 / bound
                pH = pw.tile([PT, NT, E1], F32, tag=f"pH{sfx}")
                nc.vector.tensor_tensor(
                    pH[:],
                    tint[:],
                    nrbH[:].unsqueeze(2).broadcast_to([PT, NT, E1]),
                    op=ALU.mult,
                )
                pH2 = pH[:].rearrange("p t e -> p (t e)")      # [PT,TE]

                # fp16 copy of cums_neg for the 2x-throughput accept ops
                ejh = pw.tile([PT, NT, E1], F16, tag=f"ejh{sfx}")
                nc.vector.tensor_scalar_mul(
                    ejh[:].rearrange("p t e -> p (t e)"), ej2, 1.0
                )
                ejh2 = ejh[:].rearrange("p t e -> p (t e)")

                # accept/select/reduce over ALL of u in 3 instructions:
                # d2 = pH - u*2^80 (accept>0, reject<0; |d2|>=4.8e16 always,
                # so the fp16 store saturates to EXACTLY +-inf: the fp16
                # value carries just the accept bit);
                # v = min(d2, cums_neg) in place at fp16 2x rate: accept ->
                # cums_neg in [-200,0], reject -> -inf; max-reduce picks the
                # FIRST accept (cums_neg is decreasing along e), or -inf.
                td = pb.tile([PT, K, TE], F16, tag="td")
                nc.vector.scalar_tensor_tensor(
                    td[:],
                    u_v,
                    -HUGE,
                    pH2.unsqueeze(1).broadcast_to([PT, K, TE]),
                    op0=ALU.mult,
                    op1=ALU.add,
                )
                nc.vector.tensor_tensor(
                    td[:],
                    td[:],
                    ejh2.unsqueeze(1).broadcast_to([PT, K, TE]),
                    op=ALU.min,
                )
                tred = pipe.intermediate_tile([PT, K * NT], F16, name="tred")
                nc.vector.tensor_reduce(
                    tred[:],
                    td[:].rearrange("p k (t e) -> p (k t) e", t=NT),
                    axis=AX.X,
                    op=ALU.max,
                )
                return tred

            def _store(pipe, iv, tred):
                nc.gpsimd.dma_start(ro[:, :], tred[:])

            tc.For_i_pipelined([_load, _compute, _store], 0, reps, unroll=2)

    nc.compile()
    return nc


@functools.lru_cache(maxsize=4)
def _built(reps: int):
    return _build(reps=reps)


def _host_rows(rows, e_unif, u, g_rows, muf, alf, bef):
    """Reference-faithful numpy fallback for rows not resolved within E1.

    Vectorized over all fallback rows at once (a per-row Python loop costs
    ~70 ms for the typical ~240 rows; this runs in a few ms)."""
    tn = np.linspace(0.0, DTIME_MAX, NUM_SAMPLES_BOUNDARY).astype(np.float32)
    g = np.asarray(g_rows, np.float32)[:, None, None]                 # [N,1,1]
    zb = muf[None, None, :] + alf[None, None, :] * g * np.exp(
        -bef[None, None, :] * tn[None, :, None]
    )                                                                 # [N,S,M]
    bound = (
        np.log1p(np.exp(zb)).sum(-1).max(-1) * np.float32(OVER_SAMPLE_RATE)
    ).astype(np.float32)                                              # [N]
    e = -np.log1p(-e_unif[rows])                                      # [N,E]
    expj = np.cumsum(e / bound[:, None], axis=-1).astype(np.float32)  # [N,E]
    zi = muf[None, None, :] + alf[None, None, :] * g * np.exp(
        -bef[None, None, :] * expj[:, :, None]
    )                                                                 # [N,E,M]
    it = np.log1p(np.exp(zi)).sum(-1).astype(np.float32)              # [N,E]
    crit = u[rows] * bound[:, None, None] / it[:, None, :]            # [N,K,E]
    mask = crit < 1.0
    anya = mask.any(-1)
    idx = mask.argmax(-1)                                             # [N,K]
    ej_b = np.broadcast_to(expj[:, None, :], mask.shape)
    res = np.where(
        anya, np.take_along_axis(ej_b, idx[..., None], 2)[..., 0], np.float32(0.0)
    )
    return np.minimum(res, np.float32(1.0e5)).astype(np.float32)


def kernel(
    time_seqs,
    time_delta_seqs,
    type_seqs,
    e_unif,
    u,
    mu,
    alpha,
    beta,
    gamma,
    num_sample,
    _reps: int = 1,
):
    e_unif = np.asarray(e_unif, dtype=np.float32).reshape(ROWS, E)
    u = np.asarray(u, dtype=np.float32).reshape(ROWS, K, E)
    tqf = np.ascontiguousarray(np.asarray(type_seqs).astype(np.float32)).reshape(ROWS)
    muf = np.ascontiguousarray(np.asarray(mu, dtype=np.float32))
    alf = np.ascontiguousarray(np.asarray(alpha, dtype=np.float32))
    bef = np.ascontiguousarray(np.asarray(beta, dtype=np.float32))
    gaf = np.ascontiguousarray(np.asarray(gamma, dtype=np.float32))
    arf = np.arange(NTYPES, dtype=np.float32)

    nc = _built(_reps)
    in_maps = []
    for c in range(NCORES):
        rs = slice(c * RPC, (c + 1) * RPC)
        # pack u[:, :, :E1] and eu[:, :E1]: row t*PT+p -> (p, ., t, e)
        u_part = (
            u[rs, :, :E1].reshape(NT, PT, K, E1).transpose(1, 2, 0, 3)
        )  # [PT,K,NT,E1]
        eu_part = (
            e_unif[rs, :E1].reshape(NT, PT, E1).transpose(1, 0, 2)
        )  # [PT,NT,E1]
        ui = np.concatenate(
            [u_part.reshape(PT, K, TE), eu_part.reshape(PT, 1, TE)], axis=1
        )
        in_maps.append(
            {
                "ui": np.ascontiguousarray(ui.reshape(PT, (K + 1) * TE)),
                "tq": tqf[rs],
                "mu": muf,
                "al": alf,
                "be": bef,
                "ga": gaf,
                "ar": arf,
            }
        )
    out = run_bass_kernel_spmd(nc, in_maps, core_ids=list(range(NCORES)))
    ro = np.concatenate(
        [
            out.results[c]["ro"]
            .reshape(PT, K, NT)
            .transpose(2, 0, 1)
            .reshape(RPC, K)
            for c in range(NCORES)
        ],
        axis=0,
    )  # [ROWS, K] raw max-values: -bound*exp_j at first accept, or <=-1e16

    # decode: exp_j = -val/bound (bound recomputed on host, matches device
    # within f32 rounding; only scales the output)
    g_h = gaf[tqf.astype(np.int64)]                                   # [ROWS]
    tn_h = np.linspace(0.0, DTIME_MAX, NUM_SAMPLES_BOUNDARY).astype(np.float32)
    z_h = muf[None, None, :] + alf[None, None, :] * g_h[:, None, None] * np.exp(
        -bef[None, None, :] * tn_h[None, :, None]
    )
    bound_h = (
        np.log1p(np.exp(z_h)).sum(-1).max(-1) * np.float32(OVER_SAMPLE_RATE)
    ).astype(np.float32)                                              # [ROWS]

    res = np.minimum(-ro / bound_h[:, None], np.float32(1.0e5)).astype(np.float32)
    bad_rows = np.nonzero((ro <= -BIGF).any(axis=1))[0]
    if len(bad_rows):
        res[bad_rows] = _host_rows(
            bad_rows, e_unif, u, g_h[bad_rows], muf, alf, bef
        )

    res = res.reshape(B, L, K)
    weights = np.full((B, L, K), 1.0 / float(num_sample), dtype=np.float32)
    return res, weights

